# revision 26
# baseline (speedup 1.0000x reference)
"""Low-rank causal attention on 8 TRN2 NeuronCores.

Sharding: core c -> batch b = c//4, head-group hg = c%4 (4 of 16 heads).
Per-core kernel (no collectives), chunk-interleaved so the PE stays dense:
  for each q-chunk ci (512 wide):
    proj(ci):  qkT r-tiles = P(Wqk) @ x_b[:,chunk]^T, ssq via ones-matmul,
               inv norms = exp(-0.5*ln(ssq)) (single ACT table set),
               qT=0.25*q/||q||, kT=k/||k||, v-tiles for the chunk
    attn(ci):  per k-block j (128 wide, causal; band blocks sliced to the
               live q range):
                 sT_h = kT_h x qT_h  (4 heads row-packed in the PE array)
                 heads 0-2: pT = exp(sT) in ONE merged ACT op
                 head 3:    pT = ((1+sT/4)*tri)^4 (DVE/GPSIMD, 2nd-order exp)
                 band diagonal 128x128 square masked by a universal
                 lower-triangle 0/1 const (same for every diagonal block)
                 yT[h] += v_aug_h[kblk] matmul -> [65, 512] (row 64 = denom)
Host unshard: y_head = (yT[0:64]/max(yT[64],1e-6)).T into out[b,:,head*64:+64].
"""

import os
from contextlib import ExitStack

import numpy as np
import ml_dtypes

import concourse.bass as bass
from concourse import bacc
import concourse.mybir as mybir
import concourse.tile as tile
from concourse.bass_utils import run_bass_kernel_spmd

B, N, D = 2, 2048, 1024
RANK, HEADS = 256, 16
HS = RANK // HEADS          # 16
DH = D // HEADS             # 64
NCORES = 8
HPC = 4                     # heads per core
QCH = 512                   # query chunk (free dim)
KB = 128                    # key block (partition dim)
NQC = N // QCH              # 4 query chunks
NKB = N // KB               # 16 key blocks
KTILES = D // 128           # 8 contraction tiles

F32 = mybir.dt.float32

_USE_BF16 = os.environ.get("KERNEL_DT", "bf16") == "bf16"
DT = mybir.dt.bfloat16 if _USE_BF16 else mybir.dt.float32
NPDT = ml_dtypes.bfloat16 if _USE_BF16 else np.float32
_USE_GP = os.environ.get("KERNEL_GP", "1") == "1"

_CACHE = {}
LAST_RESULT = None


def _pin_act_tables():
    """Restrict the ACT table-set choice to natural_log_exp_and_others, which
    contains every function this kernel uses (exp, ln, square, copy). Keeps
    set ids aligned with act_info.json while making the load-placement pass
    pick one set for the whole kernel -> a single ACT_TABLE_LOAD."""
    if getattr(bacc, "_act_tables_pinned", False):
        return
    orig = bacc.get_activation_tables

    def pinned(arch):
        t = orig(arch)
        return {
            k: (v if k == "natural_log_exp_and_others" else set())
            for k, v in t.items()
        }

    bacc.get_activation_tables = pinned
    bacc._act_tables_pinned = True


def _build_nc():
    if os.environ.get("KERNEL_PIN", "1") == "1":
        _pin_act_tables()
    nc = bacc.Bacc("TRN2", target_bir_lowering=False)
    xT = nc.declare_dram_parameter("xT", [D, N], DT, isOutput=False)
    wqkT = nc.declare_dram_parameter("wqkT", [D, 2 * RANK], DT, isOutput=False)
    wvT = nc.declare_dram_parameter("wvT", [D, HPC * DH], DT, isOutput=False)
    tri = nc.declare_dram_parameter("tri", [KB, KB], DT, isOutput=False)
    out = nc.declare_dram_parameter("out", [HPC * (DH + 1), N], F32, isOutput=True)

    gp = nc.gpsimd if _USE_GP else nc.vector

    with tile.TileContext(nc) as tc, ExitStack() as ctx:
        const = ctx.enter_context(tc.tile_pool(name="const", bufs=1))

        wqkT_sb = const.tile([128, KTILES, 2 * RANK], DT)
        wvT_sb = const.tile([128, KTILES, HPC * DH], DT)
        tri_sb = const.tile([KB, KB], DT)
        xT_sb = const.tile([128, KTILES, N], DT)
        nc.sync.dma_start(tri_sb[:], tri[:, :])
        # interleave so chunk 0 compute can start after ~2 tiles land
        for kk in range(KTILES):
            nc.sync.dma_start(wqkT_sb[:, kk, :], wqkT[128 * kk : 128 * kk + 128, :])
            nc.sync.dma_start(
                xT_sb[:, kk, 0:QCH], xT[128 * kk : 128 * kk + 128, 0:QCH]
            )
        for kk in range(KTILES):
            nc.sync.dma_start(wvT_sb[:, kk, :], wvT[128 * kk : 128 * kk + 128, :])
        for ci in range(1, NQC):
            for kk in range(KTILES):
                nc.sync.dma_start(
                    xT_sb[:, kk, QCH * ci : QCH * ci + QCH],
                    xT[128 * kk : 128 * kk + 128, QCH * ci : QCH * ci + QCH],
                )

        ones_sb = const.tile([128, 128], DT)
        nc.vector.memset(ones_sb[:], 1.0)

        # v with an appended ones column per head: [nk-part, ntile, head, 65]
        v_sb = const.tile([128, NKB, HPC, DH + 1], DT)
        nc.vector.memset(v_sb[:, :, :, DH : DH + 1], 1.0)

        qT_sb = const.tile([128, N], DT)   # q rows pre-scaled by 0.25/||q||
        kT_sb = const.tile([128, N], DT)   # k rows pre-scaled by 1/||k||

        for ci in range(NQC):
            ncol = slice(QCH * ci, QCH * ci + QCH)
            # ---------------- proj(ci) ----------------
            with (
                tc.tile_pool(name="qk_ps", bufs=4, space="PSUM") as qk_pool,
                tc.tile_pool(name="ss_ps", bufs=2, space="PSUM") as ss_pool,
                tc.tile_pool(name="v_ps", bufs=2, space="PSUM") as v_pool,
                tc.tile_pool(name="sq_sb", bufs=4) as sq_pool,
                tc.tile_pool(name="inv_sb", bufs=4) as inv_pool,
            ):
                qk_ps = []
                for rt in range(4):
                    ps = qk_pool.tile([128, QCH], F32)
                    qk_ps.append(ps)
                    for kk in range(KTILES):
                        nc.tensor.matmul(
                            ps[:],
                            wqkT_sb[:, kk, 128 * rt : 128 * rt + 128],
                            xT_sb[:, kk, ncol],
                            start=(kk == 0),
                            stop=(kk == KTILES - 1),
                        )
                sqs = []
                for rt in range(4):
                    sq = sq_pool.tile([128, QCH], DT, tag="sq")
                    nc.scalar.activation(
                        sq[:], qk_ps[rt][:], mybir.ActivationFunctionType.Square
                    )
                    sqs.append(sq)
                for half, scale in ((0, 16.0), (1, 1.0)):  # q: fold SCALE=0.25
                    ss = ss_pool.tile([128, QCH], F32)
                    nc.tensor.matmul(
                        ss[:], ones_sb[:], sqs[2 * half][:], start=True, stop=False
                    )
                    nc.tensor.matmul(
                        ss[:], ones_sb[:], sqs[2 * half + 1][:], start=False, stop=True
                    )
                    # 1/sqrt(scale*ss) = exp(-0.5*ln(scale*ss)); ln+exp+square
                    # share ONE ACT table set so no mid-kernel table reloads
                    lns = inv_pool.tile([128, QCH], F32, tag="lns")
                    nc.scalar.activation(
                        lns[:], ss[:], mybir.ActivationFunctionType.Ln, scale=scale
                    )
                    inv = inv_pool.tile([128, QCH], F32, tag="inv")
                    nc.scalar.activation(
                        inv[:], lns[:], mybir.ActivationFunctionType.Exp, scale=-0.5
                    )
                    if half == 0:
                        nc.vector.tensor_mul(qT_sb[:, ncol], qk_ps[0][:], inv[:])
                    else:
                        nc.vector.tensor_mul(kT_sb[:, ncol], qk_ps[2][:], inv[:])

                if ci == 0:
                    # chunk 0's v-tiles are needed immediately; later chunks'
                    # v-projections run inside the previous chunk's attention
                    # as always-ready PE filler
                    for nt in range(NQC):
                        vp = v_pool.tile([128, HPC * DH], F32)
                        for kk in range(KTILES):
                            nc.tensor.matmul(
                                vp[:],
                                xT_sb[:, kk, 128 * nt : 128 * nt + 128],
                                wvT_sb[:, kk, :],
                                start=(kk == 0),
                                stop=(kk == KTILES - 1),
                            )
                        nc.any.tensor_copy(
                            v_sb[:, nt, :, 0:DH],
                            vp[:].rearrange("p (h e) -> p h e", h=HPC),
                        )

            # ---------------- attn(ci) ----------------
            nj = NQC * ci + NQC  # causal: k-blocks 0 .. 4*ci+3
            with (
                tc.tile_pool(name="stA_ps", bufs=1, space="PSUM") as stA_pool,
                tc.tile_pool(name="stB_ps", bufs=1, space="PSUM") as stB_pool,
                tc.tile_pool(name="yt_ps", bufs=1, space="PSUM") as yt_pool,
                tc.tile_pool(name="pt_sb", bufs=4) as pt_pool,
                tc.tile_pool(name="u_sb", bufs=4) as u_pool,
                tc.tile_pool(name="yo_sb", bufs=4) as yo_pool,
            ):
                yts = [
                    yt_pool.tile([DH + 1, QCH], F32, name=f"yt{h}", tag=f"yt{h}")
                    for h in range(HPC)
                ]
                pending = []  # (j, pt, live) blocks whose yacc is deferred
                # next chunk's v-projection n-tiles, spread over this chunk's
                # blocks; they time-share the stB bank (same pool tag)
                vpend = (
                    list(range(NQC * ci + NQC, NQC * ci + 2 * NQC))
                    if ci < NQC - 1
                    else []
                )
                vstride = max(1, nj // NQC)
                for j in range(nj):
                    r = j - NQC * ci            # >= 0 on band blocks
                    qs = 128 * r if r >= 0 else 0   # live q range start
                    live = slice(qs, QCH)
                    dcol = slice(128 * r, 128 * r + 128)  # diagonal square
                    qcol = slice(QCH * ci + qs, QCH * ci + QCH)
                    stA = stA_pool.tile([128, 3, QCH], F32)
                    stB = stB_pool.tile([128, QCH], F32, tag="stB")
                    for h in range(3):
                        nc.tensor.matmul(
                            stA[:, h, live],
                            kT_sb[32 * h : 32 * h + HS, 128 * j : 128 * j + 128],
                            qT_sb[32 * h : 32 * h + HS, qcol],
                            start=True,
                            stop=True,
                            tile_position=(32 * h, 0),
                        )
                    nc.tensor.matmul(
                        stB[:, live],
                        kT_sb[96 : 96 + HS, 128 * j : 128 * j + 128],
                        qT_sb[96 : 96 + HS, qcol],
                        start=True,
                        stop=True,
                        tile_position=(96, 0),
                    )
                    pt = pt_pool.tile([128, HPC, QCH], DT)
                    # heads 0-2: one merged exp
                    nc.scalar.activation(
                        pt[:, 0:3, live],
                        stA[:, :, live],
                        mybir.ActivationFunctionType.Exp,
                    )
                    # head 3: (1 + x/4)^4 ~ exp(x) (|x| <= 0.25), mask folded
                    u = u_pool.tile([128, QCH], DT, tag="u")
                    nc.vector.tensor_scalar(
                        u[:, live], stB[:, live], 0.25, 1.0,
                        mybir.AluOpType.mult, mybir.AluOpType.add,
                    )
                    if r >= 0:
                        nc.vector.tensor_mul(u[:, dcol], u[:, dcol], tri_sb[:])
                        for h in range(3):
                            nc.vector.tensor_mul(
                                pt[:, h, dcol], pt[:, h, dcol], tri_sb[:]
                            )
                    u2 = u_pool.tile([128, QCH], DT, tag="u2")
                    nc.vector.tensor_mul(u2[:, live], u[:, live], u[:, live])
                    gp.tensor_mul(pt[:, 3, live], u2[:, live], u2[:, live])
                    # always-ready filler: next chunk's v-projection, sharing
                    # the stB bank (runs while this block's exp is on ACT)
                    if vpend and j % vstride == 0:
                        nt = vpend.pop(0)
                        vp = stB_pool.tile([128, QCH], F32, tag="stB")
                        for kk in range(KTILES):
                            nc.tensor.matmul(
                                vp[:, 0 : HPC * DH],
                                xT_sb[:, kk, 128 * nt : 128 * nt + 128],
                                wvT_sb[:, kk, :],
                                start=(kk == 0),
                                stop=(kk == KTILES - 1),
                            )
                        nc.any.tensor_copy(
                            v_sb[:, nt, :, 0:DH],
                            vp[:, 0 : HPC * DH].rearrange("p (h e) -> p h e", h=HPC),
                        )
                    # yacc lags two blocks behind: by then its pt is certainly
                    # ready, so these matmuls keep the PE fed with zero waits
                    pending.append((j, pt, live))
                    if len(pending) > 2:
                        pj, ppt, plive = pending.pop(0)
                        for h in range(HPC):
                            nc.tensor.matmul(
                                yts[h][:, plive],
                                v_sb[:, pj, h, :],
                                ppt[:, h, plive],
                                start=(pj == 0),
                                stop=False,
                            )
                for pj, ppt, plive in pending:
                    for h in range(HPC):
                        nc.tensor.matmul(
                            yts[h][:, plive],
                            v_sb[:, pj, h, :],
                            ppt[:, h, plive],
                            start=(pj == 0),
                            stop=(pj == nj - 1),
                        )
                for h in range(HPC):
                    yo = yo_pool.tile([DH + 1, QCH], F32, name=f"yo{h}", tag="yo")
                    nc.any.tensor_copy(yo[:], yts[h][:])
                    nc.sync.dma_start(
                        out[(DH + 1) * h : (DH + 1) * (h + 1), ncol], yo[:]
                    )
    nc.compile()
    return nc


def _perm_for_core(hg: int) -> np.ndarray:
    """Row permutation of Wqk: this core's q heads land at partition stripes
    32h (h=0..3) of output r-tile 0, its k heads likewise in r-tile 2."""
    perm = np.empty(2 * RANK, dtype=np.int64)
    for part, base in ((0, 0), (1, RANK)):  # q rows then k rows
        pos_used = np.zeros(RANK, dtype=bool)
        for h in range(HPC):
            head = HPC * hg + h
            rows = base + HS * head + np.arange(HS)
            perm[base + 32 * h : base + 32 * h + HS] = rows
            pos_used[32 * h : 32 * h + HS] = True
        fill_rows = [
            base + HS * head + r
            for head in range(HEADS)
            if head not in range(HPC * hg, HPC * hg + HPC)
            for r in range(HS)
        ]
        fill_pos = np.flatnonzero(~pos_used)
        perm[base + fill_pos] = fill_rows
    return perm


def kernel(x, mask, Wqk, Wv):
    global LAST_RESULT
    x = np.asarray(x)
    mask = np.asarray(mask)
    Wqk = np.asarray(Wqk)
    Wv = np.asarray(Wv)

    if "nc" not in _CACHE:
        _CACHE["nc"] = _build_nc()
    nc = _CACHE["nc"]

    # universal diagonal-square mask: keep (k <= q) within a 128x128 block
    tri01 = (np.arange(KB)[:, None] <= np.arange(KB)[None, :]).astype(NPDT)

    in_maps = []
    for c in range(NCORES):
        b, hg = divmod(c, HPC)
        perm = _perm_for_core(hg)
        in_maps.append(
            {
                "xT": np.ascontiguousarray(x[b].T).astype(NPDT),
                "wqkT": np.ascontiguousarray(Wqk[perm].T).astype(NPDT),
                "wvT": np.ascontiguousarray(
                    Wv[DH * HPC * hg : DH * HPC * (hg + 1)].T
                ).astype(NPDT),
                "tri": tri01,
            }
        )

    trace = bool(os.environ.get("KBENCH_TRACE"))
    res = run_bass_kernel_spmd(nc, in_maps, list(range(NCORES)), trace=trace)
    LAST_RESULT = res

    y = np.empty((B, N, D), dtype=np.float32)
    for c in range(NCORES):
        b, hg = divmod(c, HPC)
        arr = res.results[c]["out"]
        for h in range(HPC):
            num = arr[(DH + 1) * h : (DH + 1) * h + DH]          # [64, N]
            den = np.maximum(arr[(DH + 1) * h + DH], 1e-6)       # [N]
            head = HPC * hg + h
            y[b, :, DH * head : DH * (head + 1)] = (num / den).T
    return y


# revision 27
# speedup vs baseline: 1.0077x; 1.0077x over previous
"""Low-rank causal attention on 8 TRN2 NeuronCores.

Sharding: core c -> batch b = c//4, head-group hg = c%4 (4 of 16 heads).
Per-core kernel (no collectives), chunk-interleaved so the PE stays dense:
  for each q-chunk ci (512 wide):
    proj(ci):  qkT r-tiles = P(Wqk) @ x_b[:,chunk]^T, ssq via ones-matmul,
               inv norms = exp(-0.5*ln(ssq)) (single ACT table set),
               qT=0.25*q/||q||, kT=k/||k||, v-tiles for the chunk
    attn(ci):  per k-block j (128 wide, causal; band blocks sliced to the
               live q range):
                 sT_h = kT_h x qT_h  (4 heads row-packed in the PE array)
                 heads 0-2: pT = exp(sT) in ONE merged ACT op
                 head 3:    pT = ((1+sT/4)*tri)^4 (DVE/GPSIMD, 2nd-order exp)
                 band diagonal 128x128 square masked by a universal
                 lower-triangle 0/1 const (same for every diagonal block)
                 yT[h] += v_aug_h[kblk] matmul -> [65, 512] (row 64 = denom)
Host unshard: y_head = (yT[0:64]/max(yT[64],1e-6)).T into out[b,:,head*64:+64].
"""

import os
from contextlib import ExitStack

import numpy as np
import ml_dtypes

import concourse.bass as bass
from concourse import bacc
import concourse.mybir as mybir
import concourse.tile as tile
from concourse.bass_utils import run_bass_kernel_spmd

B, N, D = 2, 2048, 1024
RANK, HEADS = 256, 16
HS = RANK // HEADS          # 16
DH = D // HEADS             # 64
NCORES = 8
HPC = 4                     # heads per core
QCH = 512                   # query chunk (free dim)
KB = 128                    # key block (partition dim)
NQC = N // QCH              # 4 query chunks
NKB = N // KB               # 16 key blocks
KTILES = D // 128           # 8 contraction tiles

F32 = mybir.dt.float32

_USE_BF16 = os.environ.get("KERNEL_DT", "bf16") == "bf16"
DT = mybir.dt.bfloat16 if _USE_BF16 else mybir.dt.float32
NPDT = ml_dtypes.bfloat16 if _USE_BF16 else np.float32
_USE_GP = os.environ.get("KERNEL_GP", "1") == "1"

_CACHE = {}
LAST_RESULT = None


def _pin_act_tables():
    """Restrict the ACT table-set choice to natural_log_exp_and_others, which
    contains every function this kernel uses (exp, ln, square, copy). Keeps
    set ids aligned with act_info.json while making the load-placement pass
    pick one set for the whole kernel -> a single ACT_TABLE_LOAD."""
    if getattr(bacc, "_act_tables_pinned", False):
        return
    orig = bacc.get_activation_tables

    def pinned(arch):
        t = orig(arch)
        return {
            k: (v if k == "natural_log_exp_and_others" else set())
            for k, v in t.items()
        }

    bacc.get_activation_tables = pinned
    bacc._act_tables_pinned = True


def _build_nc():
    if os.environ.get("KERNEL_PIN", "1") == "1":
        _pin_act_tables()
    nc = bacc.Bacc("TRN2", target_bir_lowering=False)
    xT = nc.declare_dram_parameter("xT", [D, N], DT, isOutput=False)
    wqkT = nc.declare_dram_parameter("wqkT", [D, 2 * RANK], DT, isOutput=False)
    wvT = nc.declare_dram_parameter("wvT", [D, HPC * DH], DT, isOutput=False)
    tri = nc.declare_dram_parameter("tri", [KB, KB], DT, isOutput=False)
    out = nc.declare_dram_parameter("out", [HPC * (DH + 1), N], F32, isOutput=True)

    gp = nc.gpsimd if _USE_GP else nc.vector

    with tile.TileContext(nc) as tc, ExitStack() as ctx:
        const = ctx.enter_context(tc.tile_pool(name="const", bufs=1))

        wqkT_sb = const.tile([128, KTILES, 2 * RANK], DT)
        wvT_sb = const.tile([128, KTILES, HPC * DH], DT)
        tri_sb = const.tile([KB, KB], DT)
        xT_sb = const.tile([128, KTILES, N], DT)
        nc.sync.dma_start(tri_sb[:], tri[:, :])
        # interleave so chunk 0 compute can start after ~2 tiles land
        for kk in range(KTILES):
            nc.sync.dma_start(wqkT_sb[:, kk, :], wqkT[128 * kk : 128 * kk + 128, :])
            nc.sync.dma_start(
                xT_sb[:, kk, 0:QCH], xT[128 * kk : 128 * kk + 128, 0:QCH]
            )
        for kk in range(KTILES):
            nc.sync.dma_start(wvT_sb[:, kk, :], wvT[128 * kk : 128 * kk + 128, :])
        for ci in range(1, NQC):
            for kk in range(KTILES):
                nc.sync.dma_start(
                    xT_sb[:, kk, QCH * ci : QCH * ci + QCH],
                    xT[128 * kk : 128 * kk + 128, QCH * ci : QCH * ci + QCH],
                )

        ones_sb = const.tile([128, 128], DT)
        nc.vector.memset(ones_sb[:], 1.0)

        # v with an appended ones column per head: [nk-part, ntile, head, 65]
        v_sb = const.tile([128, NKB, HPC, DH + 1], DT)
        nc.vector.memset(v_sb[:, :, :, DH : DH + 1], 1.0)

        qT_sb = const.tile([128, N], DT)   # q rows pre-scaled by 0.25/||q||
        kT_sb = const.tile([128, N], DT)   # k rows pre-scaled by 1/||k||

        for ci in range(NQC):
            ncol = slice(QCH * ci, QCH * ci + QCH)
            # ---------------- proj(ci) ----------------
            with (
                tc.tile_pool(name="qk_ps", bufs=4, space="PSUM") as qk_pool,
                tc.tile_pool(name="ss_ps", bufs=2, space="PSUM") as ss_pool,
                tc.tile_pool(name="v_ps", bufs=2, space="PSUM") as v_pool,
                tc.tile_pool(name="sq_sb", bufs=4) as sq_pool,
                tc.tile_pool(name="inv_sb", bufs=4) as inv_pool,
            ):
                qk_ps = []
                for rt in range(4):
                    ps = qk_pool.tile([128, QCH], F32)
                    qk_ps.append(ps)
                    for kk in range(KTILES):
                        nc.tensor.matmul(
                            ps[:],
                            wqkT_sb[:, kk, 128 * rt : 128 * rt + 128],
                            xT_sb[:, kk, ncol],
                            start=(kk == 0),
                            stop=(kk == KTILES - 1),
                        )
                sqs = []
                for rt in range(4):
                    sq = sq_pool.tile([128, QCH], DT, tag="sq")
                    nc.scalar.activation(
                        sq[:], qk_ps[rt][:], mybir.ActivationFunctionType.Square
                    )
                    sqs.append(sq)
                for half, scale in ((0, 16.0), (1, 1.0)):  # q: fold SCALE=0.25
                    ss = ss_pool.tile([128, QCH], F32)
                    nc.tensor.matmul(
                        ss[:], ones_sb[:], sqs[2 * half][:], start=True, stop=False
                    )
                    nc.tensor.matmul(
                        ss[:], ones_sb[:], sqs[2 * half + 1][:], start=False, stop=True
                    )
                    # 1/sqrt(scale*ss) = exp(-0.5*ln(scale*ss)); ln+exp+square
                    # share ONE ACT table set so no mid-kernel table reloads
                    lns = inv_pool.tile([128, QCH], F32, tag="lns")
                    nc.scalar.activation(
                        lns[:], ss[:], mybir.ActivationFunctionType.Ln, scale=scale
                    )
                    inv = inv_pool.tile([128, QCH], F32, tag="inv")
                    nc.scalar.activation(
                        inv[:], lns[:], mybir.ActivationFunctionType.Exp, scale=-0.5
                    )
                    if half == 0:
                        nc.vector.tensor_mul(qT_sb[:, ncol], qk_ps[0][:], inv[:])
                    else:
                        nc.vector.tensor_mul(kT_sb[:, ncol], qk_ps[2][:], inv[:])

                for nt in range(NQC * ci, NQC * ci + NQC):
                    vp = v_pool.tile([128, HPC * DH], F32)
                    for kk in range(KTILES):
                        nc.tensor.matmul(
                            vp[:],
                            xT_sb[:, kk, 128 * nt : 128 * nt + 128],
                            wvT_sb[:, kk, :],
                            start=(kk == 0),
                            stop=(kk == KTILES - 1),
                        )
                    nc.any.tensor_copy(
                        v_sb[:, nt, :, 0:DH],
                        vp[:].rearrange("p (h e) -> p h e", h=HPC),
                    )

            # ---------------- attn(ci) ----------------
            nj = NQC * ci + NQC  # causal: k-blocks 0 .. 4*ci+3
            with (
                tc.tile_pool(name="stA_ps", bufs=1, space="PSUM") as stA_pool,
                tc.tile_pool(name="stB_ps", bufs=1, space="PSUM") as stB_pool,
                tc.tile_pool(name="yt_ps", bufs=1, space="PSUM") as yt_pool,
                tc.tile_pool(name="pt_sb", bufs=4) as pt_pool,
                tc.tile_pool(name="u_sb", bufs=4) as u_pool,
                tc.tile_pool(name="yo_sb", bufs=4) as yo_pool,
            ):
                yts = [
                    yt_pool.tile([DH + 1, QCH], F32, name=f"yt{h}", tag=f"yt{h}")
                    for h in range(HPC)
                ]
                pending = []  # (j, pt, live) blocks whose yacc is deferred
                for j in range(nj):
                    r = j - NQC * ci            # >= 0 on band blocks
                    qs = 128 * r if r >= 0 else 0   # live q range start
                    live = slice(qs, QCH)
                    dcol = slice(128 * r, 128 * r + 128)  # diagonal square
                    qcol = slice(QCH * ci + qs, QCH * ci + QCH)
                    stA = stA_pool.tile([128, 3, QCH], F32)
                    stB = stB_pool.tile([128, QCH], F32)
                    for h in range(3):
                        nc.tensor.matmul(
                            stA[:, h, live],
                            kT_sb[32 * h : 32 * h + HS, 128 * j : 128 * j + 128],
                            qT_sb[32 * h : 32 * h + HS, qcol],
                            start=True,
                            stop=True,
                            tile_position=(32 * h, 0),
                        )
                    nc.tensor.matmul(
                        stB[:, live],
                        kT_sb[96 : 96 + HS, 128 * j : 128 * j + 128],
                        qT_sb[96 : 96 + HS, qcol],
                        start=True,
                        stop=True,
                        tile_position=(96, 0),
                    )
                    pt = pt_pool.tile([128, HPC, QCH], DT)
                    # heads 0-2: one merged exp
                    nc.scalar.activation(
                        pt[:, 0:3, live],
                        stA[:, :, live],
                        mybir.ActivationFunctionType.Exp,
                    )
                    # head 3: (1 + x/4)^4 ~ exp(x) (|x| <= 0.25), mask folded
                    u = u_pool.tile([128, QCH], DT, tag="u")
                    nc.vector.tensor_scalar(
                        u[:, live], stB[:, live], 0.25, 1.0,
                        mybir.AluOpType.mult, mybir.AluOpType.add,
                    )
                    if r >= 0:
                        nc.vector.tensor_mul(u[:, dcol], u[:, dcol], tri_sb[:])
                        for h in range(3):
                            nc.vector.tensor_mul(
                                pt[:, h, dcol], pt[:, h, dcol], tri_sb[:]
                            )
                    u2 = u_pool.tile([128, QCH], DT, tag="u2")
                    nc.vector.tensor_mul(u2[:, live], u[:, live], u[:, live])
                    gp.tensor_mul(pt[:, 3, live], u2[:, live], u2[:, live])
                    # yacc lags two blocks behind: by then its pt is certainly
                    # ready, so these matmuls keep the PE fed with zero waits
                    pending.append((j, pt, live))
                    if len(pending) > 2:
                        pj, ppt, plive = pending.pop(0)
                        for h in range(HPC):
                            nc.tensor.matmul(
                                yts[h][:, plive],
                                v_sb[:, pj, h, :],
                                ppt[:, h, plive],
                                start=(pj == 0),
                                stop=False,
                            )
                for pj, ppt, plive in pending:
                    for h in range(HPC):
                        nc.tensor.matmul(
                            yts[h][:, plive],
                            v_sb[:, pj, h, :],
                            ppt[:, h, plive],
                            start=(pj == 0),
                            stop=(pj == nj - 1),
                        )
                for h in range(HPC):
                    yo = yo_pool.tile([DH + 1, QCH], F32, name=f"yo{h}", tag="yo")
                    nc.any.tensor_copy(yo[:], yts[h][:])
                    nc.sync.dma_start(
                        out[(DH + 1) * h : (DH + 1) * (h + 1), ncol], yo[:]
                    )
    nc.compile()
    return nc


def _perm_for_core(hg: int) -> np.ndarray:
    """Row permutation of Wqk: this core's q heads land at partition stripes
    32h (h=0..3) of output r-tile 0, its k heads likewise in r-tile 2."""
    perm = np.empty(2 * RANK, dtype=np.int64)
    for part, base in ((0, 0), (1, RANK)):  # q rows then k rows
        pos_used = np.zeros(RANK, dtype=bool)
        for h in range(HPC):
            head = HPC * hg + h
            rows = base + HS * head + np.arange(HS)
            perm[base + 32 * h : base + 32 * h + HS] = rows
            pos_used[32 * h : 32 * h + HS] = True
        fill_rows = [
            base + HS * head + r
            for head in range(HEADS)
            if head not in range(HPC * hg, HPC * hg + HPC)
            for r in range(HS)
        ]
        fill_pos = np.flatnonzero(~pos_used)
        perm[base + fill_pos] = fill_rows
    return perm


def kernel(x, mask, Wqk, Wv):
    global LAST_RESULT
    x = np.asarray(x)
    mask = np.asarray(mask)
    Wqk = np.asarray(Wqk)
    Wv = np.asarray(Wv)

    if "nc" not in _CACHE:
        _CACHE["nc"] = _build_nc()
    nc = _CACHE["nc"]

    # universal diagonal-square mask: keep (k <= q) within a 128x128 block
    tri01 = (np.arange(KB)[:, None] <= np.arange(KB)[None, :]).astype(NPDT)

    in_maps = []
    for c in range(NCORES):
        b, hg = divmod(c, HPC)
        perm = _perm_for_core(hg)
        in_maps.append(
            {
                "xT": np.ascontiguousarray(x[b].T).astype(NPDT),
                "wqkT": np.ascontiguousarray(Wqk[perm].T).astype(NPDT),
                "wvT": np.ascontiguousarray(
                    Wv[DH * HPC * hg : DH * HPC * (hg + 1)].T
                ).astype(NPDT),
                "tri": tri01,
            }
        )

    trace = bool(os.environ.get("KBENCH_TRACE"))
    res = run_bass_kernel_spmd(nc, in_maps, list(range(NCORES)), trace=trace)
    LAST_RESULT = res

    y = np.empty((B, N, D), dtype=np.float32)
    for c in range(NCORES):
        b, hg = divmod(c, HPC)
        arr = res.results[c]["out"]
        for h in range(HPC):
            num = arr[(DH + 1) * h : (DH + 1) * h + DH]          # [64, N]
            den = np.maximum(arr[(DH + 1) * h + DH], 1e-6)       # [N]
            head = HPC * hg + h
            y[b, :, DH * head : DH * (head + 1)] = (num / den).T
    return y


# revision 29
# speedup vs baseline: 1.0445x; 1.0366x over previous
"""Low-rank causal attention on 8 TRN2 NeuronCores.

Sharding: core c -> batch b = c//4, head-group hg = c%4 (4 of 16 heads).
Per-core kernel (no collectives), chunk-interleaved so the PE stays dense:
  for each q-chunk ci (512 wide):
    proj(ci):  qkT r-tiles = P(Wqk) @ x_b[:,chunk]^T, ssq via ones-matmul,
               inv norms = exp(-0.5*ln(ssq)) (single ACT table set),
               qT=0.25*q/||q||, kT=k/||k||, v-tiles for the chunk
    attn(ci):  per k-block j (128 wide, causal; band blocks sliced to the
               live q range):
                 sT_h = kT_h x qT_h  (4 heads row-packed in the PE array)
                 heads 0-2: pT = exp(sT) in ONE merged ACT op
                 head 3:    pT = ((1+sT/4)*tri)^4 (DVE/GPSIMD, 2nd-order exp)
                 band diagonal 128x128 square masked by a universal
                 lower-triangle 0/1 const (same for every diagonal block)
                 yT[h] += v_aug_h[kblk] matmul -> [65, 512] (row 64 = denom)
Host unshard: y_head = (yT[0:64]/max(yT[64],1e-6)).T into out[b,:,head*64:+64].
"""

import os
from contextlib import ExitStack

import numpy as np
import ml_dtypes

import concourse.bass as bass
from concourse import bacc
import concourse.mybir as mybir
import concourse.tile as tile
from concourse.bass_utils import run_bass_kernel_spmd

B, N, D = 2, 2048, 1024
RANK, HEADS = 256, 16
HS = RANK // HEADS          # 16
DH = D // HEADS             # 64
NCORES = 8
HPC = 4                     # heads per core
QCH = 512                   # query chunk (free dim)
KB = 128                    # key block (partition dim)
NQC = N // QCH              # 4 query chunks
NKB = N // KB               # 16 key blocks
KTILES = D // 128           # 8 contraction tiles

F32 = mybir.dt.float32

_USE_BF16 = os.environ.get("KERNEL_DT", "bf16") == "bf16"
DT = mybir.dt.bfloat16 if _USE_BF16 else mybir.dt.float32
NPDT = ml_dtypes.bfloat16 if _USE_BF16 else np.float32
_USE_GP = os.environ.get("KERNEL_GP", "1") == "1"

_CACHE = {}
LAST_RESULT = None


def _pin_act_tables():
    """Restrict the ACT table-set choice to natural_log_exp_and_others, which
    contains every function this kernel uses (exp, ln, square, copy). Keeps
    set ids aligned with act_info.json while making the load-placement pass
    pick one set for the whole kernel -> a single ACT_TABLE_LOAD."""
    if getattr(bacc, "_act_tables_pinned", False):
        return
    orig = bacc.get_activation_tables

    def pinned(arch):
        t = orig(arch)
        return {
            k: (v if k == "natural_log_exp_and_others" else set())
            for k, v in t.items()
        }

    bacc.get_activation_tables = pinned
    bacc._act_tables_pinned = True


def _build_nc():
    if os.environ.get("KERNEL_PIN", "1") == "1":
        _pin_act_tables()
    nc = bacc.Bacc("TRN2", target_bir_lowering=False)
    xT = nc.declare_dram_parameter("xT", [D, N], DT, isOutput=False)
    wqkT = nc.declare_dram_parameter("wqkT", [D, 2 * RANK], DT, isOutput=False)
    wvT = nc.declare_dram_parameter("wvT", [D, HPC * DH], DT, isOutput=False)
    tri = nc.declare_dram_parameter("tri", [KB, KB], DT, isOutput=False)
    out = nc.declare_dram_parameter("out", [HPC * (DH + 1), N], F32, isOutput=True)

    gp = nc.gpsimd if _USE_GP else nc.vector

    with tile.TileContext(nc) as tc, ExitStack() as ctx:
        const = ctx.enter_context(tc.tile_pool(name="const", bufs=1))

        wqkT_sb = const.tile([128, KTILES, 2 * RANK], DT)
        wvT_sb = const.tile([128, KTILES, HPC * DH], DT)
        tri_sb = const.tile([KB, KB], DT)
        xT_sb = const.tile([128, KTILES, N], DT)
        nc.sync.dma_start(tri_sb[:], tri[:, :])
        # interleave so chunk 0 compute can start after ~2 tiles land
        for kk in range(KTILES):
            nc.sync.dma_start(wqkT_sb[:, kk, :], wqkT[128 * kk : 128 * kk + 128, :])
            nc.sync.dma_start(
                xT_sb[:, kk, 0:QCH], xT[128 * kk : 128 * kk + 128, 0:QCH]
            )
        for kk in range(KTILES):
            nc.sync.dma_start(wvT_sb[:, kk, :], wvT[128 * kk : 128 * kk + 128, :])
        for ci in range(1, NQC):
            for kk in range(KTILES):
                nc.sync.dma_start(
                    xT_sb[:, kk, QCH * ci : QCH * ci + QCH],
                    xT[128 * kk : 128 * kk + 128, QCH * ci : QCH * ci + QCH],
                )

        ones_sb = const.tile([128, 128], DT)
        nc.vector.memset(ones_sb[:], 1.0)

        # v with an appended ones column per head: [nk-part, ntile, head, 65]
        v_sb = const.tile([128, NKB, HPC, DH + 1], DT)
        nc.vector.memset(v_sb[:, :, :, DH : DH + 1], 1.0)

        qT_sb = const.tile([128, N], DT)   # q rows pre-scaled by 0.25/||q||
        kT_sb = const.tile([128, N], DT)   # k rows pre-scaled by 1/||k||

        for ci in range(NQC):
            ncol = slice(QCH * ci, QCH * ci + QCH)
            # ---------------- proj(ci) ----------------
            with (
                tc.tile_pool(name="qk_ps", bufs=4, space="PSUM") as qk_pool,
                tc.tile_pool(name="ss_ps", bufs=2, space="PSUM") as ss_pool,
                tc.tile_pool(name="v_ps", bufs=2, space="PSUM") as v_pool,
                tc.tile_pool(name="sq_sb", bufs=4) as sq_pool,
                tc.tile_pool(name="inv_sb", bufs=4) as inv_pool,
            ):
                qk_ps = []
                for rt in range(4):
                    ps = qk_pool.tile([128, QCH], F32)
                    qk_ps.append(ps)
                    for kk in range(KTILES):
                        nc.tensor.matmul(
                            ps[:],
                            wqkT_sb[:, kk, 128 * rt : 128 * rt + 128],
                            xT_sb[:, kk, ncol],
                            start=(kk == 0),
                            stop=(kk == KTILES - 1),
                        )
                sqs = []
                for rt in range(4):
                    sq = sq_pool.tile([128, QCH], DT, tag="sq")
                    nc.scalar.activation(
                        sq[:], qk_ps[rt][:], mybir.ActivationFunctionType.Square
                    )
                    sqs.append(sq)
                for half, scale in ((0, 16.0), (1, 1.0)):  # q: fold SCALE=0.25
                    ss = ss_pool.tile([128, QCH], F32)
                    nc.tensor.matmul(
                        ss[:], ones_sb[:], sqs[2 * half][:], start=True, stop=False
                    )
                    nc.tensor.matmul(
                        ss[:], ones_sb[:], sqs[2 * half + 1][:], start=False, stop=True
                    )
                    # 1/sqrt(scale*ss) = exp(-0.5*ln(scale*ss)); ln+exp+square
                    # share ONE ACT table set so no mid-kernel table reloads
                    lns = inv_pool.tile([128, QCH], F32, tag="lns")
                    nc.scalar.activation(
                        lns[:], ss[:], mybir.ActivationFunctionType.Ln, scale=scale
                    )
                    inv = inv_pool.tile([128, QCH], F32, tag="inv")
                    nc.scalar.activation(
                        inv[:], lns[:], mybir.ActivationFunctionType.Exp, scale=-0.5
                    )
                    if half == 0:
                        nc.vector.tensor_mul(qT_sb[:, ncol], qk_ps[0][:], inv[:])
                    else:
                        nc.vector.tensor_mul(kT_sb[:, ncol], qk_ps[2][:], inv[:])

                for nt in range(NQC * ci, NQC * ci + NQC):
                    vp = v_pool.tile([128, HPC * DH], F32)
                    for kk in range(KTILES):
                        nc.tensor.matmul(
                            vp[:],
                            xT_sb[:, kk, 128 * nt : 128 * nt + 128],
                            wvT_sb[:, kk, :],
                            start=(kk == 0),
                            stop=(kk == KTILES - 1),
                        )
                    nc.any.tensor_copy(
                        v_sb[:, nt, :, 0:DH],
                        vp[:].rearrange("p (h e) -> p h e", h=HPC),
                    )

            # ---------------- attn(ci) ----------------
            nj = NQC * ci + NQC  # causal: k-blocks 0 .. 4*ci+3
            with (
                tc.tile_pool(name="stA_ps", bufs=1, space="PSUM") as stA_pool,
                tc.tile_pool(name="yt_ps", bufs=1, space="PSUM") as yt_pool,
                tc.tile_pool(name="pt_sb", bufs=4) as pt_pool,
                tc.tile_pool(name="yo_sb", bufs=4) as yo_pool,
            ):
                yts = [
                    yt_pool.tile([DH + 1, QCH], F32, name=f"yt{h}", tag=f"yt{h}")
                    for h in range(HPC)
                ]
                pending = []  # (j, pt, live) blocks whose yacc is deferred
                for j in range(nj):
                    r = j - NQC * ci            # >= 0 on band blocks
                    qs = 128 * r if r >= 0 else 0   # live q range start
                    live = slice(qs, QCH)
                    dcol = slice(128 * r, 128 * r + 128)  # diagonal square
                    qcol = slice(QCH * ci + qs, QCH * ci + QCH)
                    stA = stA_pool.tile([128, HPC, QCH], F32)
                    for h in range(HPC):
                        nc.tensor.matmul(
                            stA[:, h, live],
                            kT_sb[32 * h : 32 * h + HS, 128 * j : 128 * j + 128],
                            qT_sb[32 * h : 32 * h + HS, qcol],
                            start=True,
                            stop=True,
                            tile_position=(32 * h, 0),
                        )
                    pt = pt_pool.tile([128, HPC, QCH], DT)
                    # all 4 heads: one merged exp (single critical-path hop)
                    nc.scalar.activation(
                        pt[:, :, live],
                        stA[:, :, live],
                        mybir.ActivationFunctionType.Exp,
                    )
                    if r >= 0:
                        for h in range(HPC):
                            nc.vector.tensor_mul(
                                pt[:, h, dcol], pt[:, h, dcol], tri_sb[:]
                            )
                    # yacc lags two blocks behind: by then its pt is certainly
                    # ready, so these matmuls keep the PE fed with zero waits
                    pending.append((j, pt, live))
                    if len(pending) > 2:
                        pj, ppt, plive = pending.pop(0)
                        for h in range(HPC):
                            nc.tensor.matmul(
                                yts[h][:, plive],
                                v_sb[:, pj, h, :],
                                ppt[:, h, plive],
                                start=(pj == 0),
                                stop=False,
                            )
                for pj, ppt, plive in pending:
                    for h in range(HPC):
                        nc.tensor.matmul(
                            yts[h][:, plive],
                            v_sb[:, pj, h, :],
                            ppt[:, h, plive],
                            start=(pj == 0),
                            stop=(pj == nj - 1),
                        )
                for h in range(HPC):
                    yo = yo_pool.tile([DH + 1, QCH], F32, name=f"yo{h}", tag="yo")
                    nc.any.tensor_copy(yo[:], yts[h][:])
                    nc.sync.dma_start(
                        out[(DH + 1) * h : (DH + 1) * (h + 1), ncol], yo[:]
                    )
    nc.compile()
    return nc


def _perm_for_core(hg: int) -> np.ndarray:
    """Row permutation of Wqk: this core's q heads land at partition stripes
    32h (h=0..3) of output r-tile 0, its k heads likewise in r-tile 2."""
    perm = np.empty(2 * RANK, dtype=np.int64)
    for part, base in ((0, 0), (1, RANK)):  # q rows then k rows
        pos_used = np.zeros(RANK, dtype=bool)
        for h in range(HPC):
            head = HPC * hg + h
            rows = base + HS * head + np.arange(HS)
            perm[base + 32 * h : base + 32 * h + HS] = rows
            pos_used[32 * h : 32 * h + HS] = True
        fill_rows = [
            base + HS * head + r
            for head in range(HEADS)
            if head not in range(HPC * hg, HPC * hg + HPC)
            for r in range(HS)
        ]
        fill_pos = np.flatnonzero(~pos_used)
        perm[base + fill_pos] = fill_rows
    return perm


def kernel(x, mask, Wqk, Wv):
    global LAST_RESULT
    x = np.asarray(x)
    mask = np.asarray(mask)
    Wqk = np.asarray(Wqk)
    Wv = np.asarray(Wv)

    if "nc" not in _CACHE:
        _CACHE["nc"] = _build_nc()
    nc = _CACHE["nc"]

    # universal diagonal-square mask: keep (k <= q) within a 128x128 block
    tri01 = (np.arange(KB)[:, None] <= np.arange(KB)[None, :]).astype(NPDT)

    in_maps = []
    for c in range(NCORES):
        b, hg = divmod(c, HPC)
        perm = _perm_for_core(hg)
        in_maps.append(
            {
                "xT": np.ascontiguousarray(x[b].T).astype(NPDT),
                "wqkT": np.ascontiguousarray(Wqk[perm].T).astype(NPDT),
                "wvT": np.ascontiguousarray(
                    Wv[DH * HPC * hg : DH * HPC * (hg + 1)].T
                ).astype(NPDT),
                "tri": tri01,
            }
        )

    trace = bool(os.environ.get("KBENCH_TRACE"))
    res = run_bass_kernel_spmd(nc, in_maps, list(range(NCORES)), trace=trace)
    LAST_RESULT = res

    y = np.empty((B, N, D), dtype=np.float32)
    for c in range(NCORES):
        b, hg = divmod(c, HPC)
        arr = res.results[c]["out"]
        for h in range(HPC):
            num = arr[(DH + 1) * h : (DH + 1) * h + DH]          # [64, N]
            den = np.maximum(arr[(DH + 1) * h + DH], 1e-6)       # [N]
            head = HPC * hg + h
            y[b, :, DH * head : DH * (head + 1)] = (num / den).T
    return y
